# revision 20
# baseline (speedup 1.0000x reference)
"""CLUBMean loss kernel for Trainium2, 8-core data-parallel, fp16 stream.

Math: the reference loss collapses exactly (the quadratic terms cancel):
  loss = mean_i mu_i . (y_i - mean_j y_j)
       = (1/N) sum_i mu_i.y_i  -  (sum_i mu_i / N) . (sum_j y_j / N)
so the kernel only needs pooled vectors, the MLP, one covariance dot per
sample, and the two mean vectors. Samples are streamed as fp16 (host cast):
halves HBM traffic; measured end-to-end rel err ~2e-3 vs the 2e-2 gate.

Each core handles 128 of the 1024 samples:
  - sync HWDGE streams x (8 x 64ch) then y (4 x 64ch) fp16 chunks, 1 MiB per
    DMA (8 KiB/partition rows keep the SDMA packets at full efficiency);
    first/last chunks split into sub-DMAs so the pipeline starts/drains fast
  - pooling per unit = fp16 tensor_tensor fold chain 64->32->16->8 (2x DVE
    rate; op count amortized over 64ch) + one f32 tensor_reduce (1x);
    GpSimd takes the level-1 fold on a few x chunks to share load
  - DVE back-to-back ops shorter than the ~420ns pipe-drain window do NOT
    interlock on HW (CoreSim's race detector is right): any such producer->
    consumer edge is semaphore-chained (s_dch)
  - PE transposes pooled x (f32), ACT scale-copies (1/64) to fp16; MLP runs
    as fp16 matmuls into f32 PSUM; mu is back-transposed to sample-major
  - dot blocks D_b = sum_c mu[n,c]*yv[n,c] via sem-chained mul+reduce
  - outputs: yv (pooled y, unscaled; host sums for the y-mean), Mu, D blocks

Host combine (f64): loss = sum(D)/64/N - (Mu/N).(sum(yv)/64/N).
Each DMA's +16 semaphore arrives as +1 per DGE lane; chunk completion uses
one semaphore per transfer.
"""

import sys

sys.path.insert(0, "/opt/trn_rl_repo")

from contextlib import ExitStack

import numpy as np

import concourse.bass as bass
import concourse.mybir as mybir
from concourse.bass_utils import run_bass_kernel_spmd

N = 1024
P = 128            # samples per core
XC, YC, HID, S = 512, 256, 512, 64
CH = 64            # channels per streamed chunk (1 MiB fp16)
NBUF = 12          # stream buffers (one per chunk, no reuse)
NF = 4             # fold chain buffer ring (unit-indexed)
WCOLS = 3072       # fp16 weight pack: w1 (4k x 512h) | w2 (4k x 256c)
F32 = mybir.dt.float32
F16 = mybir.dt.float16
AX = mybir.AxisListType
ALU = mybir.AluOpType
ACTF = mybir.ActivationFunctionType

# ---- chunk / pool-unit tables ----------------------------------------------
# chunks: (is_y, c0) with CH channels each. x: 8 (ids 0-7), y: 4 (ids 8-11).
CHUNKS = [(0, c * CH) for c in range(8)] + [(1, c * CH) for c in range(4)]
NCHUNK = len(CHUNKS)       # 12
# stream order: y chunks 8,9 ride mid-x (GpSimd pre-folds them; drains the
# y tail)
ORDER = [0, 1, 2, 8, 3, 4, 9, 5, 6, 7, 10, 11]
GCHUNKS = (8, 9)           # chunks whose level-1 fold runs on GpSimd
# per-chunk sub-DMA channel ranges (each is one DMA + one pool unit)
SUBS = {0: [(0, 32), (32, 64)],
        11: [(0, 48), (48, 56), (56, 64)]}
WPOS = 3                   # weights ride the stream after this many DMAs

UNITS = []                 # (chunk, lo, hi)
for c in ORDER:
    for lo, hi in SUBS.get(c, [(0, CH)]):
        UNITS.append((c, lo, hi))
NU = len(UNITS)            # 15
ULAST = {}
for u, (c, lo, hi) in enumerate(UNITS):
    ULAST[c] = u
GUNITS = [u for u, (c, lo, hi) in enumerate(UNITS) if c in GCHUNKS]
GRANK = {u: r for r, u in enumerate(GUNITS)}

# y column blocks for the dot + output DMAs (yv columns)
POOL_A, POOL_B, POOL_C = ULAST[9] + 1, ULAST[10] + 1, ULAST[11] + 1
# pool-unit counts after which xv column block k (128 cols) is complete
XVB = [max(ULAST[2 * k], ULAST[2 * k + 1]) + 1 for k in range(4)]

DEBUG = False

_CACHE = {}


def build_nc(debug=False):
    nc = bass.Bass()
    x = nc.dram_tensor("x", [P, XC, S], F16, kind="ExternalInput")
    y = nc.dram_tensor("y", [P, YC, S], F16, kind="ExternalInput")
    wpack = nc.dram_tensor("wpack", [P, WCOLS], F16, kind="ExternalInput")
    bias = nc.dram_tensor("bias", [P, 8], F32, kind="ExternalInput")
    ident_in = nc.dram_tensor("ident_in", [P, P], F32, kind="ExternalInput")
    out_yv = nc.dram_tensor("out_yv", [P, YC], F32, kind="ExternalOutput")
    out_stat = nc.dram_tensor("out_stat", [P, 2], F32, kind="ExternalOutput")
    out_d = nc.dram_tensor("out_d", [P, 2], F32, kind="ExternalOutput")
    if debug:
        dbg_muN = nc.dram_tensor("dbg_muN", [P, YC], F32, kind="ExternalOutput")
        dbg_dacc = nc.dram_tensor("dbg_dacc", [P, 4], F32, kind="ExternalOutput")

    ctx = ExitStack()
    with ctx:
        sb = lambda name, shape, dt=F32: ctx.enter_context(
            nc.sbuf_tensor(name, shape, dt)
        )
        ps = lambda name, shape: ctx.enter_context(nc.psum_tensor(name, shape, F32))
        sem = lambda name: ctx.enter_context(nc.semaphore(name))

        xbuf = sb("xbuf", [P, NBUF, CH, S], F16)
        f1 = sb("f1", [P, NF, CH, 32], F16)
        f2 = sb("f2", [P, NF, CH, 16], F16)
        f3 = sb("f3", [P, NF, CH, 8], F16)
        xv = sb("xv", [P, XC])
        yv = sb("yv", [P, YC])
        wsb = sb("wsb", [P, WCOLS], F16)
        bsb = sb("bsb", [P, 8])
        xvT = sb("xvT", [P, 4, P], F16)
        hT = sb("hT", [P, 4, P], F16)
        muT = sb("muT", [P, 2, P])
        muN = sb("muN", [P, YC])
        stat2 = sb("stat2", [P, 2])
        dacc = sb("dacc", [P, 4])
        scr = sb("scr", [P, YC])
        ident = sb("ident", [P, P])

        pt = [ps(f"pt{i}", [P, P]) for i in range(2)]
        ph = [ps(f"ph{i}", [P, P]) for i in range(4)]
        pmu = [ps(f"pmu{i}", [P, P]) for i in range(2)]

        dsem = [sem(f"d{u}") for u in range(NU)]
        dw = sem("dw")
        dout = sem("dout")
        s_pool = sem("s_pool")
        s_gfold = sem("s_gfold")
        s_tp = sem("s_tp")
        s_cp = sem("s_cp")
        s_hmm = sem("s_hmm")
        s_relu = sem("s_relu")
        s_mumm = sem("s_mumm")
        s_mucp = sem("s_mucp")
        s_tpmu = sem("s_tpmu")
        s_mun = sem("s_mun")
        s_stat = sem("s_stat")
        s_ttr = sem("s_ttr")
        s_dch = sem("s_dch")

        def chunk_src(c, lo, hi):
            is_y, c0 = CHUNKS[c]
            t = y if is_y else x
            return t[:, c0 + lo:c0 + hi, :]

        def pool_dst(u):
            c, lo, hi = UNITS[u]
            is_y, c0 = CHUNKS[c]
            t = yv if is_y else xv
            return t[:, c0 + lo:c0 + hi]

        with nc.Block() as block:

            @block.sync
            def _(e):
                for u, (c, lo, hi) in enumerate(UNITS):
                    e.dma_start(
                        out=xbuf[:, c % NBUF, lo:hi, :], in_=chunk_src(c, lo, hi)
                    ).then_inc(dsem[u], 16)
                e.wait_ge(s_ttr, 2)
                e.dma_start(out=out_d[:, :], in_=dacc[:, 0:2]).then_inc(dout, 16)
                e.wait_ge(dout, 80 + (32 if debug else 0))

            @block.gpsimd
            def _(e):
                for u in GUNITS:
                    c, lo, hi = UNITS[u]
                    e.wait_ge(dsem[u], 16)
                    if u >= NF:
                        e.wait_ge(s_pool, u - NF + 1)
                    e.tensor_add(
                        f1[:, u % NF, 0:hi - lo, :],
                        xbuf[:, c % NBUF, lo:hi, 0:32],
                        xbuf[:, c % NBUF, lo:hi, 32:64],
                    ).then_inc(s_gfold, 1)

            @block.vector
            def _(e):
                dch = [0]

                def chain(inst):
                    # sem-enforce a short-op RAW edge on the DVE
                    dch[0] += 1
                    inst.then_inc(s_dch, 1)
                    e.wait_ge(s_dch, dch[0])

                def pool_unit(u):
                    c, lo, hi = UNITS[u]
                    w = hi - lo
                    s = u % NF
                    if u in GRANK:
                        e.wait_ge(s_gfold, GRANK[u] + 1)
                    else:
                        e.wait_ge(dsem[u], 16)
                        inst = e.tensor_add(
                            f1[:, s, 0:w, :],
                            xbuf[:, c % NBUF, lo:hi, 0:32],
                            xbuf[:, c % NBUF, lo:hi, 32:64],
                        )
                        if w <= 8:
                            # ~300ns op feeding the reduce: chain the RAW edge
                            chain(inst)
                    if w >= 64:
                        e.tensor_add(
                            f2[:, s, 0:w, :], f1[:, s, 0:w, 0:16],
                            f1[:, s, 0:w, 16:32],
                        )
                        e.tensor_add(
                            f3[:, s, 0:w, :], f2[:, s, 0:w, 0:8],
                            f2[:, s, 0:w, 8:16],
                        )
                        red_in = f3[:, s, 0:w, :]
                    elif w >= 32:
                        e.tensor_add(
                            f2[:, s, 0:w, :], f1[:, s, 0:w, 0:16],
                            f1[:, s, 0:w, 16:32],
                        )
                        red_in = f2[:, s, 0:w, :]
                    else:
                        red_in = f1[:, s, 0:w, :]
                    e.tensor_reduce(
                        pool_dst(u), red_in, axis=AX.X, op=ALU.add
                    ).then_inc(s_pool, 1)

                def dot(blk, cols0, cols1):
                    chain(e.tensor_mul(
                        scr[:, cols0:cols1],
                        muN[:, cols0:cols1],
                        yv[:, cols0:cols1],
                    ))
                    e.tensor_reduce(
                        dacc[:, blk:blk + 1],
                        scr[:, cols0:cols1],
                        axis=AX.X,
                        op=ALU.add,
                    ).then_inc(s_ttr, 1)

                for u in range(NU):
                    pool_unit(u)
                    if u == ULAST[10]:
                        e.wait_ge(s_mucp, 2)
                        e.tensor_reduce(
                            stat2[:, 0:1], muT[:, 0, :], axis=AX.X, op=ALU.add
                        )
                        e.tensor_reduce(
                            stat2[:, 1:2], muT[:, 1, :], axis=AX.X, op=ALU.add
                        ).then_inc(s_stat, 1)
                        e.wait_ge(s_mun, 2)
                        dot(0, 0, 192)
                dot(1, 192, 256)

            @block.tensor
            def _(e):
                e.wait_ge(dw, 48)
                # pipeline with the stream: per k-block, transpose xv cols,
                # then (after ACT copies it to fp16) accumulate that k-block
                # into each h output's own PSUM tensor
                for k, need in enumerate((XVB[0], XVB[1], XVB[2], XVB[3])):
                    e.wait_ge(s_pool, need)
                    if k >= 2:
                        e.wait_ge(s_cp, k - 1)
                    e.transpose(
                        pt[k % 2][:, :], xv[:, k * P:(k + 1) * P], ident[:, :]
                    ).then_inc(s_tp, 1)
                    e.wait_ge(s_cp, k + 1)
                    for m in range(4):
                        mm = e.matmul(
                            ph[m][:, :],
                            wsb[:, k * HID + m * P:k * HID + (m + 1) * P],
                            xvT[:, k, :],
                            start=(k == 0),
                            stop=(k == 3),
                        )
                mm.then_inc(s_hmm, 1)
                for k in range(4):
                    e.wait_ge(s_relu, k + 1)
                    for m in range(2):
                        mm = e.matmul(
                            pmu[m][:, :],
                            wsb[:, 2048 + k * YC + m * P:2048 + k * YC + (m + 1) * P],
                            hT[:, k, :],
                            start=(k == 0),
                            stop=(k == 3),
                        )
                mm.then_inc(s_mumm, 1)
                e.wait_ge(s_mucp, 2)
                for m in range(2):
                    e.transpose(pt[m][:, :], muT[:, m, :], ident[:, :]).then_inc(
                        s_tpmu, 1
                    )

            @block.scalar
            def _(e):
                e.dma_start(out=wsb[:, :], in_=wpack[:, :]).then_inc(dw, 16)
                e.dma_start(out=bsb[:, :], in_=bias[:, :]).then_inc(dw, 16)
                e.dma_start(out=ident[:, :], in_=ident_in[:, :]).then_inc(dw, 16)
                for m in range(4):
                    e.wait_ge(s_tp, m + 1)
                    e.activation(
                        xvT[:, m, :], pt[m % 2][:, :], ACTF.Copy, scale=1.0 / S
                    ).then_inc(s_cp, 1)
                e.wait_ge(s_hmm, 1)
                for m in range(4):
                    e.activation(
                        hT[:, m, :], ph[m][:, :], ACTF.Relu, bias=bsb[:, m:m + 1]
                    ).then_inc(s_relu, 1)
                e.wait_ge(s_mumm, 1)
                for m in range(2):
                    e.activation(
                        muT[:, m, :], pmu[m][:, :], ACTF.Identity,
                        bias=bsb[:, 4 + m:5 + m],
                    ).then_inc(s_mucp, 1)
                for m in range(2):
                    e.wait_ge(s_tpmu, m + 1)
                    e.activation(
                        muN[:, m * P:(m + 1) * P], pt[m][:, :], ACTF.Copy
                    ).then_inc(s_mun, 1)
                e.wait_ge(s_stat, 1)
                e.dma_start(out=out_stat[:, :], in_=stat2[:, :]).then_inc(dout, 16)
                e.wait_ge(s_pool, POOL_A)
                e.dma_start(out=out_yv[:, 0:128], in_=yv[:, 0:128]).then_inc(dout, 16)
                e.wait_ge(s_pool, POOL_B)
                e.dma_start(out=out_yv[:, 128:192], in_=yv[:, 128:192]).then_inc(
                    dout, 16
                )
                e.wait_ge(s_pool, POOL_C)
                e.dma_start(out=out_yv[:, 192:256], in_=yv[:, 192:256]).then_inc(
                    dout, 16
                )
                if debug:
                    e.dma_start(out=dbg_muN[:, :], in_=muN[:, :]).then_inc(dout, 16)
                    e.dma_start(out=dbg_dacc[:, :], in_=dacc[:, :]).then_inc(dout, 16)

    return nc


def _get_nc():
    if "nc" not in _CACHE:
        _CACHE["nc"] = build_nc(debug=DEBUG)
    return _CACHE["nc"]


def make_in_maps(x_samples, y_samples, W1, b1, W2, b2):
    xs = np.asarray(x_samples, np.float32).reshape(N, XC, S).astype(np.float16)
    ys = np.asarray(y_samples, np.float32).reshape(N, YC, S).astype(np.float16)
    wp = np.zeros((P, WCOLS), np.float16)
    wp[:, :2048] = (
        np.asarray(W1, np.float16).reshape(4, P, HID).transpose(1, 0, 2).reshape(P, 2048)
    )
    wp[:, 2048:3072] = (
        np.asarray(W2, np.float16).reshape(4, P, YC).transpose(1, 0, 2).reshape(P, 1024)
    )
    wp = np.ascontiguousarray(wp)
    bp = np.zeros((P, 8), np.float32)
    bp[:, 0:4] = np.asarray(b1, np.float32).reshape(4, P).T
    bp[:, 4:6] = np.asarray(b2, np.float32).reshape(2, P).T
    bp = np.ascontiguousarray(bp)
    idm = np.ascontiguousarray(np.eye(P, dtype=np.float32))
    in_maps = []
    for c in range(8):
        in_maps.append(
            {
                "x": np.ascontiguousarray(xs[c * P:(c + 1) * P]),
                "y": np.ascontiguousarray(ys[c * P:(c + 1) * P]),
                "wpack": wp,
                "bias": bp,
                "ident_in": idm,
            }
        )
    return in_maps


def combine(results):
    dot = 0.0
    Mu = np.zeros(YC, np.float64)
    Ey = np.zeros(YC, np.float64)
    for c in range(8):
        stat = results[c]["out_stat"].astype(np.float64)   # (128, 2)
        yvc = results[c]["out_yv"].astype(np.float64)      # (128, 256)
        dot += results[c]["out_d"].astype(np.float64).sum()
        Mu += np.concatenate([stat[:, 0], stat[:, 1]])
        Ey += yvc.sum(axis=0)
    dot /= S
    Ey /= S
    loss = dot / N - float((Mu / N) @ (Ey / N))
    return np.float32(loss)


def run(inputs, **kwargs):
    nc = _get_nc()
    in_maps = make_in_maps(**inputs)
    res = run_bass_kernel_spmd(nc, in_maps, core_ids=list(range(8)), **kwargs)
    return combine(res.results), res


def kernel(x_samples, y_samples, W1, b1, W2, b2):
    loss, _ = run(
        dict(
            x_samples=x_samples,
            y_samples=y_samples,
            W1=W1,
            b1=b1,
            W2=W2,
            b2=b2,
        )
    )
    return loss


# revision 21
# speedup vs baseline: 1.3196x; 1.3196x over previous
"""CLUBMean loss kernel for Trainium2, 8-core data-parallel, fp16 stream.

Math: the reference loss collapses exactly (the quadratic terms cancel):
  loss = mean_i mu_i . (y_i - mean_j y_j)
       = (1/N) sum_i mu_i.y_i  -  (sum_i mu_i / N) . (sum_j y_j / N)
so the kernel only needs pooled vectors, the MLP, one covariance dot per
sample, and the two mean vectors. Samples are streamed as fp16 (host cast):
halves HBM traffic; measured end-to-end rel err ~2e-3 vs the 2e-2 gate.

Each core handles 128 of the 1024 samples:
  - sync HWDGE streams x (8 x 64ch) then y (4 x 64ch) fp16 chunks, 1 MiB per
    DMA (8 KiB/partition rows keep the SDMA packets at full efficiency);
    first/last chunks split into sub-DMAs so the pipeline starts/drains fast
  - pooling per unit = fp16 tensor_tensor fold chain 64->32->16->8 (2x DVE
    rate; op count amortized over 64ch) + one f32 tensor_reduce (1x);
    GpSimd takes the level-1 fold on a few x chunks to share load
  - DVE back-to-back ops shorter than the ~420ns pipe-drain window do NOT
    interlock on HW (CoreSim's race detector is right): any such producer->
    consumer edge is semaphore-chained (s_dch)
  - PE transposes pooled x (f32), ACT scale-copies (1/64) to fp16; MLP runs
    as fp16 matmuls into f32 PSUM; mu is back-transposed to sample-major
  - dot blocks D_b = sum_c mu[n,c]*yv[n,c] via sem-chained mul+reduce
  - outputs: yv (pooled y, unscaled; host sums for the y-mean), Mu, D blocks

Host combine (f64): loss = sum(D)/64/N - (Mu/N).(sum(yv)/64/N).
Each DMA's +16 semaphore arrives as +1 per DGE lane; chunk completion uses
one semaphore per transfer.
"""

import sys

sys.path.insert(0, "/opt/trn_rl_repo")

from contextlib import ExitStack

import numpy as np

import concourse.bass as bass
import concourse.mybir as mybir
from concourse.bass_utils import run_bass_kernel_spmd

N = 1024
P = 128            # samples per core
XC, YC, HID, S = 512, 256, 512, 64
CH = 64            # channels per streamed chunk (1 MiB fp16)
NBUF = 12          # stream buffers (one per chunk, no reuse)
NF = 4             # fold chain buffer ring (unit-indexed)
WCOLS = 3072       # fp16 weight pack: w1 (4k x 512h) | w2 (4k x 256c)
F32 = mybir.dt.float32
F16 = mybir.dt.float16
AX = mybir.AxisListType
ALU = mybir.AluOpType
ACTF = mybir.ActivationFunctionType

# ---- chunk / pool-unit tables ----------------------------------------------
# chunks: (is_y, c0) with CH channels each. x: 8 (ids 0-7), y: 4 (ids 8-11).
CHUNKS = [(0, c * CH) for c in range(8)] + [(1, c * CH) for c in range(4)]
NCHUNK = len(CHUNKS)       # 12
# stream order: y chunk 8 rides mid-x (fills DVE slack, drains the y tail)
ORDER = [0, 1, 2, 3, 4, 5, 8, 6, 7, 9, 10, 11]
GCHUNKS = ()               # gp folds hurt: Q7 adds lock the DVE's SBUF port
# per-chunk sub-DMA channel ranges (each is one DMA + one pool unit)
SUBS = {0: [(0, 32), (32, 64)],
        11: [(0, 48), (48, 56), (56, 64)]}
WPOS = 3                   # weights ride the stream after this many DMAs

UNITS = []                 # (chunk, lo, hi)
for c in ORDER:
    for lo, hi in SUBS.get(c, [(0, CH)]):
        UNITS.append((c, lo, hi))
NU = len(UNITS)            # 15
ULAST = {}
for u, (c, lo, hi) in enumerate(UNITS):
    ULAST[c] = u
GUNITS = [u for u, (c, lo, hi) in enumerate(UNITS) if c in GCHUNKS]
GRANK = {u: r for r, u in enumerate(GUNITS)}

# y column blocks for the dot + output DMAs (yv columns)
POOL_A, POOL_B, POOL_C = ULAST[9] + 1, ULAST[10] + 1, ULAST[11] + 1
# pool-unit counts after which xv column block k (128 cols) is complete
XVB = [max(ULAST[2 * k], ULAST[2 * k + 1]) + 1 for k in range(4)]

DEBUG = False

_CACHE = {}


def build_nc(debug=False):
    nc = bass.Bass()
    x = nc.dram_tensor("x", [P, XC, S], F16, kind="ExternalInput")
    y = nc.dram_tensor("y", [P, YC, S], F16, kind="ExternalInput")
    wpack = nc.dram_tensor("wpack", [P, WCOLS], F16, kind="ExternalInput")
    bias = nc.dram_tensor("bias", [P, 8], F32, kind="ExternalInput")
    ident_in = nc.dram_tensor("ident_in", [P, P], F32, kind="ExternalInput")
    out_yv = nc.dram_tensor("out_yv", [P, YC], F32, kind="ExternalOutput")
    out_stat = nc.dram_tensor("out_stat", [P, 2], F32, kind="ExternalOutput")
    out_d = nc.dram_tensor("out_d", [P, 2], F32, kind="ExternalOutput")
    if debug:
        dbg_muN = nc.dram_tensor("dbg_muN", [P, YC], F32, kind="ExternalOutput")
        dbg_dacc = nc.dram_tensor("dbg_dacc", [P, 4], F32, kind="ExternalOutput")

    ctx = ExitStack()
    with ctx:
        sb = lambda name, shape, dt=F32: ctx.enter_context(
            nc.sbuf_tensor(name, shape, dt)
        )
        ps = lambda name, shape: ctx.enter_context(nc.psum_tensor(name, shape, F32))
        sem = lambda name: ctx.enter_context(nc.semaphore(name))

        xbuf = sb("xbuf", [P, NBUF, CH, S], F16)
        f1 = sb("f1", [P, NF, CH, 32], F16)
        f2 = sb("f2", [P, NF, CH, 16], F16)
        f3 = sb("f3", [P, NF, CH, 8], F16)
        xv = sb("xv", [P, XC])
        yv = sb("yv", [P, YC])
        wsb = sb("wsb", [P, WCOLS], F16)
        bsb = sb("bsb", [P, 8])
        xvT = sb("xvT", [P, 4, P], F16)
        hT = sb("hT", [P, 4, P], F16)
        muT = sb("muT", [P, 2, P])
        muN = sb("muN", [P, YC])
        stat2 = sb("stat2", [P, 2])
        dacc = sb("dacc", [P, 4])
        scr = sb("scr", [P, YC])
        ident = sb("ident", [P, P])

        pt = [ps(f"pt{i}", [P, P]) for i in range(2)]
        ph = [ps(f"ph{i}", [P, P]) for i in range(4)]
        pmu = [ps(f"pmu{i}", [P, P]) for i in range(2)]

        dsem = [sem(f"d{u}") for u in range(NU)]
        dw = sem("dw")
        dout = sem("dout")
        s_pool = sem("s_pool")
        s_gfold = sem("s_gfold")
        s_tp = sem("s_tp")
        s_cp = sem("s_cp")
        s_hmm = sem("s_hmm")
        s_relu = sem("s_relu")
        s_mumm = sem("s_mumm")
        s_mucp = sem("s_mucp")
        s_tpmu = sem("s_tpmu")
        s_mun = sem("s_mun")
        s_stat = sem("s_stat")
        s_ttr = sem("s_ttr")
        s_dch = sem("s_dch")

        def chunk_src(c, lo, hi):
            is_y, c0 = CHUNKS[c]
            t = y if is_y else x
            return t[:, c0 + lo:c0 + hi, :]

        def pool_dst(u):
            c, lo, hi = UNITS[u]
            is_y, c0 = CHUNKS[c]
            t = yv if is_y else xv
            return t[:, c0 + lo:c0 + hi]

        with nc.Block() as block:

            @block.sync
            def _(e):
                for u, (c, lo, hi) in enumerate(UNITS):
                    e.dma_start(
                        out=xbuf[:, c % NBUF, lo:hi, :], in_=chunk_src(c, lo, hi)
                    ).then_inc(dsem[u], 16)
                e.wait_ge(s_ttr, 2)
                e.dma_start(out=out_d[:, :], in_=dacc[:, 0:2]).then_inc(dout, 16)
                e.wait_ge(dout, 80 + (32 if debug else 0))

            @block.vector
            def _(e):
                dch = [0]

                def chain(inst):
                    # sem-enforce a short-op RAW edge on the DVE
                    dch[0] += 1
                    inst.then_inc(s_dch, 1)
                    e.wait_ge(s_dch, dch[0])

                def pool_unit(u):
                    c, lo, hi = UNITS[u]
                    w = hi - lo
                    s = u % NF
                    if u in GRANK:
                        e.wait_ge(s_gfold, GRANK[u] + 1)
                    else:
                        e.wait_ge(dsem[u], 16)
                        inst = e.tensor_add(
                            f1[:, s, 0:w, :],
                            xbuf[:, c % NBUF, lo:hi, 0:32],
                            xbuf[:, c % NBUF, lo:hi, 32:64],
                        )
                        if w <= 8:
                            # ~300ns op feeding the reduce: chain the RAW edge
                            chain(inst)
                    if w >= 64:
                        e.tensor_add(
                            f2[:, s, 0:w, :], f1[:, s, 0:w, 0:16],
                            f1[:, s, 0:w, 16:32],
                        )
                        e.tensor_add(
                            f3[:, s, 0:w, :], f2[:, s, 0:w, 0:8],
                            f2[:, s, 0:w, 8:16],
                        )
                        red_in = f3[:, s, 0:w, :]
                    elif w >= 32:
                        e.tensor_add(
                            f2[:, s, 0:w, :], f1[:, s, 0:w, 0:16],
                            f1[:, s, 0:w, 16:32],
                        )
                        red_in = f2[:, s, 0:w, :]
                    else:
                        red_in = f1[:, s, 0:w, :]
                    e.tensor_reduce(
                        pool_dst(u), red_in, axis=AX.X, op=ALU.add
                    ).then_inc(s_pool, 1)

                def dot(blk, cols0, cols1):
                    chain(e.tensor_mul(
                        scr[:, cols0:cols1],
                        muN[:, cols0:cols1],
                        yv[:, cols0:cols1],
                    ))
                    e.tensor_reduce(
                        dacc[:, blk:blk + 1],
                        scr[:, cols0:cols1],
                        axis=AX.X,
                        op=ALU.add,
                    ).then_inc(s_ttr, 1)

                for u in range(NU):
                    pool_unit(u)
                    if u == ULAST[10]:
                        e.wait_ge(s_mucp, 2)
                        e.tensor_reduce(
                            stat2[:, 0:1], muT[:, 0, :], axis=AX.X, op=ALU.add
                        )
                        e.tensor_reduce(
                            stat2[:, 1:2], muT[:, 1, :], axis=AX.X, op=ALU.add
                        ).then_inc(s_stat, 1)
                        e.wait_ge(s_mun, 2)
                        dot(0, 0, 192)
                dot(1, 192, 256)

            @block.tensor
            def _(e):
                e.wait_ge(dw, 48)
                # pipeline with the stream: per k-block, transpose xv cols,
                # then (after ACT copies it to fp16) accumulate that k-block
                # into each h output's own PSUM tensor
                for k, need in enumerate((XVB[0], XVB[1], XVB[2], XVB[3])):
                    e.wait_ge(s_pool, need)
                    if k >= 2:
                        e.wait_ge(s_cp, k - 1)
                    e.transpose(
                        pt[k % 2][:, :], xv[:, k * P:(k + 1) * P], ident[:, :]
                    ).then_inc(s_tp, 1)
                    e.wait_ge(s_cp, k + 1)
                    for m in range(4):
                        mm = e.matmul(
                            ph[m][:, :],
                            wsb[:, k * HID + m * P:k * HID + (m + 1) * P],
                            xvT[:, k, :],
                            start=(k == 0),
                            stop=(k == 3),
                        )
                mm.then_inc(s_hmm, 1)
                for k in range(4):
                    e.wait_ge(s_relu, k + 1)
                    for m in range(2):
                        mm = e.matmul(
                            pmu[m][:, :],
                            wsb[:, 2048 + k * YC + m * P:2048 + k * YC + (m + 1) * P],
                            hT[:, k, :],
                            start=(k == 0),
                            stop=(k == 3),
                        )
                mm.then_inc(s_mumm, 1)
                e.wait_ge(s_mucp, 2)
                for m in range(2):
                    e.transpose(pt[m][:, :], muT[:, m, :], ident[:, :]).then_inc(
                        s_tpmu, 1
                    )

            @block.scalar
            def _(e):
                e.dma_start(out=wsb[:, :], in_=wpack[:, :]).then_inc(dw, 16)
                e.dma_start(out=bsb[:, :], in_=bias[:, :]).then_inc(dw, 16)
                e.dma_start(out=ident[:, :], in_=ident_in[:, :]).then_inc(dw, 16)
                for m in range(4):
                    e.wait_ge(s_tp, m + 1)
                    e.activation(
                        xvT[:, m, :], pt[m % 2][:, :], ACTF.Copy, scale=1.0 / S
                    ).then_inc(s_cp, 1)
                e.wait_ge(s_hmm, 1)
                for m in range(4):
                    e.activation(
                        hT[:, m, :], ph[m][:, :], ACTF.Relu, bias=bsb[:, m:m + 1]
                    ).then_inc(s_relu, 1)
                e.wait_ge(s_mumm, 1)
                for m in range(2):
                    e.activation(
                        muT[:, m, :], pmu[m][:, :], ACTF.Identity,
                        bias=bsb[:, 4 + m:5 + m],
                    ).then_inc(s_mucp, 1)
                for m in range(2):
                    e.wait_ge(s_tpmu, m + 1)
                    e.activation(
                        muN[:, m * P:(m + 1) * P], pt[m][:, :], ACTF.Copy
                    ).then_inc(s_mun, 1)
                e.wait_ge(s_stat, 1)
                e.dma_start(out=out_stat[:, :], in_=stat2[:, :]).then_inc(dout, 16)
                e.wait_ge(s_pool, POOL_A)
                e.dma_start(out=out_yv[:, 0:128], in_=yv[:, 0:128]).then_inc(dout, 16)
                e.wait_ge(s_pool, POOL_B)
                e.dma_start(out=out_yv[:, 128:192], in_=yv[:, 128:192]).then_inc(
                    dout, 16
                )
                e.wait_ge(s_pool, POOL_C)
                e.dma_start(out=out_yv[:, 192:256], in_=yv[:, 192:256]).then_inc(
                    dout, 16
                )
                if debug:
                    e.dma_start(out=dbg_muN[:, :], in_=muN[:, :]).then_inc(dout, 16)
                    e.dma_start(out=dbg_dacc[:, :], in_=dacc[:, :]).then_inc(dout, 16)

    return nc


def _get_nc():
    if "nc" not in _CACHE:
        _CACHE["nc"] = build_nc(debug=DEBUG)
    return _CACHE["nc"]


def make_in_maps(x_samples, y_samples, W1, b1, W2, b2):
    xs = np.asarray(x_samples, np.float32).reshape(N, XC, S).astype(np.float16)
    ys = np.asarray(y_samples, np.float32).reshape(N, YC, S).astype(np.float16)
    wp = np.zeros((P, WCOLS), np.float16)
    wp[:, :2048] = (
        np.asarray(W1, np.float16).reshape(4, P, HID).transpose(1, 0, 2).reshape(P, 2048)
    )
    wp[:, 2048:3072] = (
        np.asarray(W2, np.float16).reshape(4, P, YC).transpose(1, 0, 2).reshape(P, 1024)
    )
    wp = np.ascontiguousarray(wp)
    bp = np.zeros((P, 8), np.float32)
    bp[:, 0:4] = np.asarray(b1, np.float32).reshape(4, P).T
    bp[:, 4:6] = np.asarray(b2, np.float32).reshape(2, P).T
    bp = np.ascontiguousarray(bp)
    idm = np.ascontiguousarray(np.eye(P, dtype=np.float32))
    in_maps = []
    for c in range(8):
        in_maps.append(
            {
                "x": np.ascontiguousarray(xs[c * P:(c + 1) * P]),
                "y": np.ascontiguousarray(ys[c * P:(c + 1) * P]),
                "wpack": wp,
                "bias": bp,
                "ident_in": idm,
            }
        )
    return in_maps


def combine(results):
    dot = 0.0
    Mu = np.zeros(YC, np.float64)
    Ey = np.zeros(YC, np.float64)
    for c in range(8):
        stat = results[c]["out_stat"].astype(np.float64)   # (128, 2)
        yvc = results[c]["out_yv"].astype(np.float64)      # (128, 256)
        dot += results[c]["out_d"].astype(np.float64).sum()
        Mu += np.concatenate([stat[:, 0], stat[:, 1]])
        Ey += yvc.sum(axis=0)
    dot /= S
    Ey /= S
    loss = dot / N - float((Mu / N) @ (Ey / N))
    return np.float32(loss)


def run(inputs, **kwargs):
    nc = _get_nc()
    in_maps = make_in_maps(**inputs)
    res = run_bass_kernel_spmd(nc, in_maps, core_ids=list(range(8)), **kwargs)
    return combine(res.results), res


def kernel(x_samples, y_samples, W1, b1, W2, b2):
    loss, _ = run(
        dict(
            x_samples=x_samples,
            y_samples=y_samples,
            W1=W1,
            b1=b1,
            W2=W2,
            b2=b2,
        )
    )
    return loss
